# revision 1
# baseline (speedup 1.0000x reference)
"""FP8-weight dense linear (FFN up-proj) on 8 Trainium2 NeuronCores.

Computes out[128, 16384] = x[128, 4096] @ dequant(weight_fp8[16384, 4096]).T
+ bias, tensor-parallel: weight/bias sharded along out_features (2048 rows
per core), x replicated, output gathered by concatenation (no collectives).

Per-core kernel design (v7):
- The PE contracts over the partition dim, so the fp8 weight needs
  in_features on partitions. The HW xbar DMA-transpose only supports
  2-byte elements, so the weight shard is viewed as uint16 (adjacent fp8
  pairs along in_features). One >=8MB transpose per iteration reaches the
  xbar ceiling (~280 GB/s); the Tile scheduler serializes transposes
  against every other DMA (completion-chained, both directions - the HW
  deadlock guard), so each extra DMA in the loop costs its full drain +
  receipt latency in the serialized stream.
- Therefore x RIDES THE WEIGHT TRANSPOSE: the host appends a
  column-permuted fp16 copy of x (256 extra uint16 rows) to the weight
  shard, so the single 9MB xbar transpose delivers the fp8 weights
  [in_features on partitions] AND x^T slots [128, slot, t] in one
  transfer. The only remaining aux DMAs are the 0.25MB output store and
  the 4KB bias load.
- The For_i body is unrolled 8x: within a body, the MMs of sub-iteration
  p overlap sub-iteration p+1's transpose stream (wt double-buffered);
  the all-engine barrier + sem-reset that For_i inserts per body is
  amortized over 8 sub-iterations.
- Output stores are lagged: two persistent fp16 out_sb buffers
  (ping-pong): sub-iteration p stores buffer (p%2)'s contents (written 2
  sub-iterations ago, evictions long complete) then evicts new results
  into it; an epilogue store flushes the final sub-iteration. DVE evicts
  PSUM fp32 -> SBUF fp16 (halves store bytes); the host upcasts.
- Bias (fp16) enters as a rank-1 matmul (ones[1,128].T @ bias[1,512])
  that opens each PSUM accumulation group.
"""

import sys

if "/opt/trn_rl_repo" not in sys.path:
    sys.path.insert(0, "/opt/trn_rl_repo")

import numpy as np

import concourse.bass as bass  # noqa: F401  (registers bass lowering)
import concourse.mybir as mybir
import concourse.tile as tile
from concourse import bacc
from concourse.bass_utils import run_bass_kernel_spmd  # noqa: F401

N_CORES = 8
T = 128          # tokens
K = 4096         # in_features
O_FULL = 16384   # out_features
O = O_FULL // N_CORES  # 2048 per core
O_CHUNK = 512    # psum bank / matmul free dim
N_OSL = O // O_CHUNK   # 4 o-slices per core
JT = K // 2 // 128     # 16 pair-tiles (each covers 256 in_features)
XROWS = 2 * T          # x rows appended to the weight shard (u16 view)
WX = O + XROWS         # combined tensor rows per core (2304)
N_SLAB = 2             # o-slabs per iteration (4MB-class transfers sustain
                       # ~268 GB/s in-system; one 9MB transfer only ~251)
SLAB = WX // N_SLAB    # rows per slab (1152 = 1024 weight + 128 x rows)
OSLAB = O // N_SLAB    # weight rows per slab (1024)

_NC = None


def _build_nc(repeats: int = 1, wbufs: int = 2, psum_bufs: int = 8,
              unroll: int = 8):
    nc = bacc.Bacc("TRN2", target_bir_lowering=False, debug=False,
                   num_devices=N_CORES)
    w_d = nc.dram_tensor("wxt", [WX, K // 2], mybir.dt.uint16,
                         kind="ExternalInput")
    b_d = nc.dram_tensor("bias", [1, O], mybir.dt.float16,
                         kind="ExternalInput")
    # out holds TWO copies of the result (the store unit is a pair
    # tile); the host reads columns [0, O)
    o_d = nc.dram_tensor("out", [T, 2 * O], mybir.dt.float16,
                         kind="ExternalOutput")

    with tile.TileContext(nc) as tc:
        with (
            tc.tile_pool(name="const", bufs=1) as const,
            tc.tile_pool(name="bpool", bufs=2) as bpool,
            tc.tile_pool(name="wpool", bufs=wbufs) as wpool,
            tc.tile_pool(name="psum", bufs=psum_bufs, space="PSUM") as psum,
        ):
            ones = const.tile([1, T], mybir.dt.float16)
            nc.any.memset(ones[:], 1.0)
            # persistent pair-tile output staging: result of sub-iter g
            # is evicted into half g%2 of pair tile (g//2)%2. At even p
            # ONE dma stores the pair holding results p-4 and p-3 (both
            # eviction-complete, so the store never chains the in-flight
            # MM tail into the serialized DMA stream), and the aux block
            # runs only every 2nd sub-iteration. Both halves hold the
            # same values (every iteration recomputes the same result).
            out_sbs = []
            for q in range(2):
                osb = const.tile([T, 2 * O], mybir.dt.float16,
                                 name=f"out_pair{q}")
                nc.any.memset(osb[:], 0.0)
                out_sbs.append(osb)

            def body(p, bias_sb=None):
                out_tile = out_sbs[(p // 2) % 2]
                out_half = (p % 2) * O
                # aux DMA (every 2nd sub-iteration): one lagged
                # pair-store on the same sync ring before the transposes
                if p % 2 == 0:
                    nc.sync.dma_start(o_d.ap(),
                                      out_sbs[((p - 4) // 2) % 2][:])
                if bias_sb is None:
                    bias_sb = bpool.tile([1, O], mybir.dt.float16,
                                         name=f"bias{p}", tag="bias")
                    nc.sync.dma_start(bias_sb[:], b_d.ap())

                # two combined-slab transposes, back-to-back on the ring.
                # Slab h: weight o-rows [1024h, 1024h+1024) + the x rows
                # for slots c in [16h, 16h+16).
                wts = []
                for h in range(N_SLAB):
                    wt = wpool.tile([128, JT, SLAB], mybir.dt.uint16,
                                    name=f"w{p}_{h}", tag="wt")
                    nc.sync.dma_start(
                        wt[:], w_d.ap()[h * SLAB:(h + 1) * SLAB, :],
                        transpose=True)
                    wts.append(wt)
                wt8s = [w[:].bitcast(mybir.dt.float8e4) for w in wts]
                wtfs = [w[:].bitcast(mybir.dt.float16) for w in wts]

                def lhs_of(c):
                    return wtfs[c // JT][:, c % JT, OSLAB:OSLAB + T]

                def mm(osl, jt, par):
                    c = 2 * jt + par
                    rhs = wt8s[osl // 2][:, jt, par::2][
                        :, (osl % 2) * O_CHUNK:(osl % 2 + 1) * O_CHUNK]
                    nc.tensor.matmul(
                        ps_tiles[osl][:], lhs_of(c), rhs, start=False,
                        stop=(jt == JT - 1 and par == 1))

                ps_tiles = {}

                def open_group(osl):
                    ps = psum.tile([T, O_CHUNK], mybir.dt.float32,
                                   name=f"ps{p}_{osl}", tag="ps")
                    nc.tensor.matmul(
                        ps[:], ones[:],
                        bias_sb[:, osl * O_CHUNK:(osl + 1) * O_CHUNK],
                        start=True, stop=False)
                    ps_tiles[osl] = ps

                def close_group(osl):
                    off = out_half + osl * O_CHUNK
                    nc.vector.tensor_copy(
                        out_tile[:, off:off + O_CHUNK], ps_tiles[osl][:])

                # phase A (needs only slab 0): groups 0/1, x-slots 0..15
                body_ret[0] = bias_sb
                for osl in (0, 1):
                    open_group(osl)
                    for jt in range(JT // 2):
                        for par in (0, 1):
                            mm(osl, jt, par)
                # phase B (needs slab 1 too)
                for osl in (0, 1):
                    for jt in range(JT // 2, JT):
                        for par in (0, 1):
                            mm(osl, jt, par)
                    close_group(osl)
                for osl in (2, 3):
                    open_group(osl)
                    for jt in range(JT):
                        for par in (0, 1):
                            mm(osl, jt, par)
                    close_group(osl)

            body_ret = [None]
            if repeats == 1:
                body(0)
                last = 0
            else:
                assert repeats % unroll == 0
                with tc.For_i(0, repeats // unroll, 1):
                    bias_sb = None
                    for p in range(unroll):
                        body(p, bias_sb)
                        bias_sb = body_ret[0]
                last = unroll - 1
            # epilogue: flush the pair holding the final result
            nc.sync.dma_start(o_d.ap(), out_sbs[(last // 2) % 2][:])

    nc.compile()
    return nc


BEST_CONFIG = dict(wbufs=4, psum_bufs=8, unroll=12)


def _get_nc():
    global _NC
    if _NC is None:
        _NC = _build_nc(**BEST_CONFIG)
    return _NC


def make_per_core_inputs(x, weight_fp8, bias):
    """Host-side shard/layout prep shared by kernel() and the timing
    harness. Returns {name: array} with the per-core concatenated layout
    expected by the SPMD callable (axis 0 sharded over cores)."""
    x = np.ascontiguousarray(np.asarray(x), dtype=np.float32)
    w = np.ascontiguousarray(np.asarray(weight_fp8))
    b = np.ascontiguousarray(np.asarray(bias), dtype=np.float32)
    assert x.shape == (T, K) and w.shape == (O_FULL, K)

    w16 = w.view(np.uint16)  # [16384, 2048]
    # xh[t, c*128 + p] = fp16(x[t, 256*(c//2) + 2*p + (c%2)]), c=2*jt+par
    x16 = x.astype(np.float16)
    xh = np.ascontiguousarray(
        x16.reshape(T, JT, 128, 2).transpose(0, 1, 3, 2).reshape(T, K))
    xhu = xh.view(np.uint16)
    # xh2[h2*128 + t, :] = xhu[t, h2*2048:(h2+1)*2048]
    xh2 = np.ascontiguousarray(
        xhu.reshape(T, 2, K // 2).transpose(1, 0, 2).reshape(XROWS, K // 2))

    blocks = []
    for core in range(N_CORES):
        for h in range(N_SLAB):
            blocks.append(w16[core * O + h * OSLAB:core * O + (h + 1) * OSLAB])
            blocks.append(xh2[h * T:(h + 1) * T])
    return {
        "wxt": np.concatenate(blocks, axis=0),  # [8*2304, 2048]
        "bias": b.astype(np.float16).reshape(N_CORES, O),
    }


_FN = None


def _get_fn():
    """Cache the jitted SPMD callable so repeat kernel() calls skip the
    ~1.3s of re-tracing that run_bass_kernel_spmd pays per invocation."""
    global _FN
    if _FN is not None:
        return _FN
    import jax
    from jax.sharding import Mesh, PartitionSpec, NamedSharding
    from jax.experimental.shard_map import shard_map
    from concourse import bass2jax as b2j

    nc = _get_nc()
    b2j.install_neuronx_cc_hook()
    pname = nc.partition_id_tensor.name if nc.partition_id_tensor else None
    in_names, out_names, out_avals = [], [], []
    for alloc in nc.m.functions[0].allocations:
        if not isinstance(alloc, mybir.MemoryLocationSet):
            continue
        name = alloc.memorylocations[0].name
        if alloc.kind == "ExternalInput":
            if name != pname:
                in_names.append(name)
        elif alloc.kind == "ExternalOutput":
            out_names.append(name)
            out_avals.append(jax.core.ShapedArray(
                tuple(alloc.tensor_shape), mybir.dt.np(alloc.dtype)))
    n_params, n_outs = len(in_names), len(out_avals)
    all_in = in_names + out_names + ([pname] if pname else [])

    def _body(*args):
        operands = list(args)
        if pname:
            operands.append(b2j.partition_id_tensor())
        outs = b2j._bass_exec_p.bind(
            *operands, out_avals=tuple(out_avals), in_names=tuple(all_in),
            out_names=tuple(out_names), lowering_input_output_aliases=(),
            sim_require_finite=True, sim_require_nnan=True, nc=nc)
        return tuple(outs)

    mesh = Mesh(np.asarray(jax.devices()[:N_CORES]), ("core",))
    fn = jax.jit(shard_map(_body, mesh=mesh,
                           in_specs=(PartitionSpec("core"),) * (n_params + n_outs),
                           out_specs=(PartitionSpec("core"),) * n_outs,
                           check_rep=False), keep_unused=True)
    sharding = NamedSharding(mesh, PartitionSpec("core"))
    _FN = (fn, in_names, out_avals, sharding)
    return _FN


def kernel(x, weight_fp8, bias):
    import jax

    fn, in_names, out_avals, sharding = _get_fn()
    per_core = make_per_core_inputs(x, weight_fp8, bias)
    dev_in = [jax.device_put(per_core[n], sharding) for n in in_names]
    dev_zero = [jax.device_put(
        np.zeros((N_CORES * a.shape[0], *a.shape[1:]), a.dtype), sharding)
        for a in out_avals]
    outs = fn(*dev_in, *dev_zero)
    res = np.asarray(jax.device_get(outs[0])).reshape(N_CORES, T, 2 * O)
    return np.concatenate(
        [res[c, :, :O] for c in range(N_CORES)], axis=1).astype(np.float32)



# revision 2
# speedup vs baseline: 1.1199x; 1.1199x over previous
"""FP8-weight dense linear (FFN up-proj) on 8 Trainium2 NeuronCores.

Computes out[128, 16384] = x[128, 4096] @ dequant(weight_fp8[16384, 4096]).T
+ bias, tensor-parallel: weight/bias sharded along out_features (2048 rows
per core), x replicated, output gathered by concatenation (no collectives).

Per-core kernel design (v8):
- The PE contracts over the partition dim, so both operands need
  in_features on partitions. Instead of the HW xbar DMA-transpose (~261
  GB/s ceiling, serialized against every other DMA by the deadlock
  guard), the HOST pre-transposes the fp8 weight shard to K-major
  [128, KT, O] layout, so the kernel issues plain contiguous DMA loads
  that run at the ~358 GB/s per-core HBM limit and overlap freely.
- Weight shard is loaded as two 4.2MB half-K slabs (double-buffered
  across iterations, bufs=4): per iteration DMA is 8.39MB weight +
  0.5MB fp16 output store ~= 24.9us; PE work is 32x4 matmuls of 512
  moving fp8 rows = 65536 cycles ~= 27.3us @2.4GHz, so steady-state is
  PE-bound with DMA fully hidden.
- x^T (fp16, host-pretransposed to [k_sub, kt, t]) and bias are loaded
  once before the repeat loop. Bias is pre-broadcast to all 128
  partitions via 4 rank-1 PE matmuls at startup; per iteration the DVE
  adds it during the PSUM->SBUF fp16 eviction (tensor_add), keeping
  bias off the PE critical path.
- 4 PSUM banks accumulate o-chunks of 512 across all 32 k-tiles
  (j-outer order so 4 consecutive matmuls share one stationary x^T
  tile); psum bufs=8 lets consecutive iterations overlap. Output store
  goes on the ACT HWDGE ring, weight loads on the SP ring.
"""

import sys

if "/opt/trn_rl_repo" not in sys.path:
    sys.path.insert(0, "/opt/trn_rl_repo")

import numpy as np

import concourse.bass as bass  # noqa: F401  (registers bass lowering)
import concourse.mybir as mybir
import concourse.tile as tile
from concourse import bacc
from concourse.bass_utils import run_bass_kernel_spmd  # noqa: F401

N_CORES = 8
T = 128          # tokens
K = 4096         # in_features
O_FULL = 16384   # out_features
O = O_FULL // N_CORES  # 2048 per core
O_CHUNK = 512    # psum bank / matmul free dim
N_OSL = O // O_CHUNK   # 4 o-slices per core
KT = K // 128    # 32 k-tiles of 128 contraction rows
KH = 2           # weight loaded as KH half-K slabs per iteration
KTH = KT // KH   # k-tiles per slab (16)

_NC = None


def _build_nc(repeats: int = 1, wbufs: int = 4, psum_bufs: int = 8,
              unroll: int = 12):
    nc = bacc.Bacc("TRN2", target_bir_lowering=False, debug=False,
                   num_devices=N_CORES)
    w_d = nc.dram_tensor("wt", [128, KT * O], mybir.dt.uint8,
                         kind="ExternalInput")
    x_d = nc.dram_tensor("xt", [128, KT * T], mybir.dt.float16,
                         kind="ExternalInput")
    b_d = nc.dram_tensor("bias", [1, O], mybir.dt.float16,
                         kind="ExternalInput")
    o_d = nc.dram_tensor("out", [T, O], mybir.dt.float16,
                         kind="ExternalOutput")

    with tile.TileContext(nc) as tc:
        with (
            tc.tile_pool(name="const", bufs=1) as const,
            tc.tile_pool(name="wpool", bufs=wbufs) as wpool,
            tc.tile_pool(name="opool", bufs=2) as opool,
            tc.tile_pool(name="psum", bufs=psum_bufs, space="PSUM") as psum,
        ):
            # ---- startup (outside the repeat loop) ----
            ones = const.tile([1, T], mybir.dt.float16)
            nc.any.memset(ones[:], 1.0)
            xt_sb = const.tile([128, KT, T], mybir.dt.float16)
            nc.sync.dma_start(xt_sb[:], x_d.ap())
            bias_sb = const.tile([1, O], mybir.dt.float16)
            nc.sync.dma_start(bias_sb[:], b_d.ap())
            # broadcast bias to all 128 partitions (rank-1 matmuls)
            bias_bc = const.tile([T, O], mybir.dt.float16)
            for c in range(N_OSL):
                pb = psum.tile([T, O_CHUNK], mybir.dt.float32,
                               name=f"pbias{c}", tag="ps")
                nc.tensor.matmul(
                    pb[:], ones[:],
                    bias_sb[:, c * O_CHUNK:(c + 1) * O_CHUNK],
                    start=True, stop=True)
                nc.vector.tensor_copy(
                    bias_bc[:, c * O_CHUNK:(c + 1) * O_CHUNK], pb[:])

            def body(p):
                # weight: KH half-K slabs, plain contiguous DMA
                whs = []
                for h in range(KH):
                    wh = wpool.tile([128, KTH, O], mybir.dt.uint8,
                                    name=f"w{p}_{h}", tag="wt")
                    nc.sync.dma_start(
                        wh[:], w_d.ap()[:, h * KTH * O:(h + 1) * KTH * O])
                    whs.append(wh)
                w8s = [wh[:].bitcast(mybir.dt.float8e4) for wh in whs]

                out_sb = opool.tile([T, O], mybir.dt.float16,
                                    name=f"o{p}", tag="out")
                ps = [psum.tile([T, O_CHUNK], mybir.dt.float32,
                                name=f"ps{p}_{c}", tag="ps")
                      for c in range(N_OSL)]
                for j in range(KT):
                    lhs = xt_sb[:, j, :]
                    rhs_slab = w8s[j // KTH]
                    jj = j % KTH
                    for c in range(N_OSL):
                        nc.tensor.matmul(
                            ps[c][:], lhs,
                            rhs_slab[:, jj, c * O_CHUNK:(c + 1) * O_CHUNK],
                            start=(j == 0), stop=(j == KT - 1))
                for c in range(N_OSL):
                    sl = slice(c * O_CHUNK, (c + 1) * O_CHUNK)
                    nc.vector.tensor_add(out_sb[:, sl], ps[c][:],
                                         bias_bc[:, sl])
                # store on the ACT HWDGE ring (weight loads own the SP ring)
                nc.scalar.dma_start(o_d.ap(), out_sb[:])

            if repeats == 1:
                body(0)
            else:
                assert repeats % unroll == 0
                with tc.For_i(0, repeats // unroll, 1):
                    for p in range(unroll):
                        body(p)

    nc.compile()
    return nc


BEST_CONFIG = dict(wbufs=4, psum_bufs=8, unroll=12)


def _get_nc():
    global _NC
    if _NC is None:
        _NC = _build_nc(**BEST_CONFIG)
    return _NC


def make_per_core_inputs(x, weight_fp8, bias):
    """Host-side shard/layout prep shared by kernel() and the timing
    harness. Returns {name: array} with the per-core concatenated layout
    expected by the SPMD callable (axis 0 sharded over cores)."""
    x = np.ascontiguousarray(np.asarray(x), dtype=np.float32)
    w = np.ascontiguousarray(np.asarray(weight_fp8))
    b = np.ascontiguousarray(np.asarray(bias), dtype=np.float32)
    assert x.shape == (T, K) and w.shape == (O_FULL, K)

    # weight: per core c, wt[p, j*O + o] = w[c*O + o, j*128 + p]
    w8 = w.view(np.uint8)                       # [16384, 4096]
    wt = np.ascontiguousarray(
        w8.reshape(N_CORES, O, KT, 128).transpose(0, 3, 2, 1)
    ).reshape(N_CORES * 128, KT * O)

    # x^T: xt[p, j*T + t] = fp16(x[t, j*128 + p]); replicated per core
    x16 = x.astype(np.float16)                  # [128, 4096] (t, i)
    xt1 = np.ascontiguousarray(
        x16.reshape(T, KT, 128).transpose(2, 1, 0)).reshape(128, KT * T)
    xt = np.tile(xt1, (N_CORES, 1))             # [8*128, 4096]

    return {
        "wt": wt,
        "xt": xt,
        "bias": b.astype(np.float16).reshape(N_CORES, O),
    }


_FN = None


def _get_fn():
    """Cache the jitted SPMD callable so repeat kernel() calls skip the
    ~1.3s of re-tracing that run_bass_kernel_spmd pays per invocation."""
    global _FN
    if _FN is not None:
        return _FN
    import jax
    from jax.sharding import Mesh, PartitionSpec, NamedSharding
    from jax.experimental.shard_map import shard_map
    from concourse import bass2jax as b2j

    nc = _get_nc()
    b2j.install_neuronx_cc_hook()
    pname = nc.partition_id_tensor.name if nc.partition_id_tensor else None
    in_names, out_names, out_avals = [], [], []
    for alloc in nc.m.functions[0].allocations:
        if not isinstance(alloc, mybir.MemoryLocationSet):
            continue
        name = alloc.memorylocations[0].name
        if alloc.kind == "ExternalInput":
            if name != pname:
                in_names.append(name)
        elif alloc.kind == "ExternalOutput":
            out_names.append(name)
            out_avals.append(jax.core.ShapedArray(
                tuple(alloc.tensor_shape), mybir.dt.np(alloc.dtype)))
    n_params, n_outs = len(in_names), len(out_avals)
    all_in = in_names + out_names + ([pname] if pname else [])

    def _body(*args):
        operands = list(args)
        if pname:
            operands.append(b2j.partition_id_tensor())
        outs = b2j._bass_exec_p.bind(
            *operands, out_avals=tuple(out_avals), in_names=tuple(all_in),
            out_names=tuple(out_names), lowering_input_output_aliases=(),
            sim_require_finite=True, sim_require_nnan=True, nc=nc)
        return tuple(outs)

    mesh = Mesh(np.asarray(jax.devices()[:N_CORES]), ("core",))
    fn = jax.jit(shard_map(_body, mesh=mesh,
                           in_specs=(PartitionSpec("core"),) * (n_params + n_outs),
                           out_specs=(PartitionSpec("core"),) * n_outs,
                           check_rep=False), keep_unused=True)
    sharding = NamedSharding(mesh, PartitionSpec("core"))
    _FN = (fn, in_names, out_avals, sharding)
    return _FN


def kernel(x, weight_fp8, bias):
    import jax

    fn, in_names, out_avals, sharding = _get_fn()
    per_core = make_per_core_inputs(x, weight_fp8, bias)
    dev_in = [jax.device_put(per_core[n], sharding) for n in in_names]
    dev_zero = [jax.device_put(
        np.zeros((N_CORES * a.shape[0], *a.shape[1:]), a.dtype), sharding)
        for a in out_avals]
    outs = fn(*dev_in, *dev_zero)
    res = np.asarray(jax.device_get(outs[0])).reshape(N_CORES, T, O)
    return np.concatenate(
        [res[c] for c in range(N_CORES)], axis=1).astype(np.float32)


# revision 18
# speedup vs baseline: 1.4810x; 1.3225x over previous
"""FP8-weight dense linear (FFN up-proj) on 8 Trainium2 NeuronCores.

Computes out[128, 16384] = x[128, 4096] @ dequant(weight_fp8[16384, 4096]).T
+ bias, tensor-parallel: weight/bias sharded along out_features (2048 rows
per core), x replicated, output gathered by concatenation (no collectives).

Per-core kernel design (v8):
- The PE contracts over the partition dim, so both operands need
  in_features on partitions. Instead of the HW xbar DMA-transpose (~261
  GB/s ceiling, serialized against every other DMA by the deadlock
  guard), the HOST pre-transposes the fp8 weight shard to K-major
  [128, KT, O] layout, so the kernel issues plain contiguous DMA loads
  that run at the ~358 GB/s per-core HBM limit and overlap freely.
- Weight shard is loaded as two 4.2MB half-K slabs (double-buffered
  across iterations, bufs=4): per iteration DMA is 8.39MB weight +
  0.5MB fp16 output store ~= 24.9us; PE work is 32x4 matmuls of 512
  moving fp8 rows = 65536 cycles ~= 27.3us @2.4GHz, so steady-state is
  PE-bound with DMA fully hidden.
- x^T (fp16, host-pretransposed to [k_sub, kt, t]) and bias are loaded
  once before the repeat loop. Bias is pre-broadcast to all 128
  partitions via 4 rank-1 PE matmuls at startup; per iteration the DVE
  adds it during the PSUM->SBUF fp16 eviction (tensor_add), keeping
  bias off the PE critical path.
- 4 PSUM banks accumulate o-chunks of 512 across all 32 k-tiles
  (j-outer order so 4 consecutive matmuls share one stationary x^T
  tile); psum bufs=8 lets consecutive iterations overlap. Output store
  goes on the ACT HWDGE ring, weight loads on the SP ring.
"""

import sys

if "/opt/trn_rl_repo" not in sys.path:
    sys.path.insert(0, "/opt/trn_rl_repo")

import numpy as np

import concourse.bass as bass  # noqa: F401  (registers bass lowering)
import concourse.mybir as mybir
import concourse.tile as tile
from concourse import bacc
from concourse.bass_utils import run_bass_kernel_spmd  # noqa: F401

N_CORES = 8
T = 128          # tokens
K = 4096         # in_features
O_FULL = 16384   # out_features
O = O_FULL // N_CORES  # 2048 per core
O_CHUNK = 512    # psum bank / matmul free dim
N_OSL = O // O_CHUNK   # 4 o-slices per core
KT = K // 128    # 32 k-tiles of 128 contraction rows
KH = 4           # weight loaded as KH slabs of K per iteration
KTH = KT // KH   # k-tiles per slab (16)
NK8 = 14         # trailing k-tiles done as fp8 DoubleRow (2 tiles/matmul)

_NC = None


def _build_nc(repeats: int = 1, wbufs: int = 4, psum_bufs: int = 8,
              unroll: int = 12, variant: str = "full",
              hint: bool = False, stag: bool = False, kh: int = KH,
              nk8: int = NK8):
    nc = bacc.Bacc("TRN2", target_bir_lowering=False, debug=False,
                   num_devices=N_CORES)
    w_d = nc.dram_tensor("wt", [128, KT * O], mybir.dt.uint8,
                         kind="ExternalInput")
    x_d = nc.dram_tensor("xt", [128, KT * T], mybir.dt.float16,
                         kind="ExternalInput")
    x8_d = nc.dram_tensor("xt8", [128, max(nk8, 2) * T], mybir.dt.uint8,
                          kind="ExternalInput")
    b_d = nc.dram_tensor("bias", [1, O], mybir.dt.float16,
                         kind="ExternalInput")
    o_d = nc.dram_tensor("out", [T, O], mybir.dt.float16,
                         kind="ExternalOutput")

    with tile.TileContext(nc) as tc:
        with (
            tc.tile_pool(name="const", bufs=1) as const,
            tc.tile_pool(name="wpool", bufs=wbufs) as wpool,
            tc.tile_pool(name="opool", bufs=2) as opool,
            tc.tile_pool(name="psum", bufs=psum_bufs, space="PSUM") as psum,
        ):
            # ---- startup (outside the repeat loop) ----
            ones = const.tile([1, T], mybir.dt.float16)
            nc.any.memset(ones[:], 1.0)
            xt_sb = const.tile([128, KT, T], mybir.dt.float16)
            nc.sync.dma_start(xt_sb[:], x_d.ap())
            xt8_sb = const.tile([128, max(nk8, 2) // 2, 2, T],
                                mybir.dt.uint8)
            nc.sync.dma_start(xt8_sb[:], x8_d.ap())
            xt8f = xt8_sb[:].bitcast(mybir.dt.float8e4)
            bias_sb = const.tile([1, O], mybir.dt.float16)
            nc.sync.dma_start(bias_sb[:], b_d.ap())
            # broadcast bias to all 128 partitions (rank-1 matmuls)
            bias_bc = const.tile([T, O], mybir.dt.float16)
            for c in range(N_OSL):
                pb = psum.tile([T, O_CHUNK], mybir.dt.float32,
                               name=f"pbias{c}", tag="ps")
                nc.tensor.matmul(
                    pb[:], ones[:],
                    bias_sb[:, c * O_CHUNK:(c + 1) * O_CHUNK],
                    start=True, stop=True)
                nc.vector.tensor_copy(
                    bias_bc[:, c * O_CHUNK:(c + 1) * O_CHUNK], pb[:])

            def body(p):
                # weight: kh slabs of K, plain contiguous DMA
                kth_full = KT // kh
                kth = 1 if "smalldma" in variant else kth_full
                whs = []
                for h in range(kh):
                    wh = wpool.tile([128, kth, O], mybir.dt.uint8,
                                    name=f"w{p}_{h}", tag="wt")
                    nc.sync.dma_start(
                        wh[:],
                        w_d.ap()[:, h * kth_full * O:h * kth_full * O
                                 + kth * O])
                    whs.append(wh)
                w8s = [wh[:].bitcast(mybir.dt.float8e4) for wh in whs]

                out_sb = opool.tile([T, O], mybir.dt.float16,
                                    name=f"o{p}", tag="out")
                ps = [psum.tile([T, O_CHUNK], mybir.dt.float32,
                                name=f"ps{p}_{c}", tag="ps")
                      for c in range(N_OSL)]
                if variant != "nomm":
                    kt_eff = KT // 2 if "halfk" in variant else KT
                    nc_eff = N_OSL // 2 if "halfc" in variant else N_OSL
                    nsplit = 2 if "halfn" in variant else 1
                    oc = O_CHUNK // nsplit
                    kt16 = kt_eff - (nk8 if kt_eff == KT else 0)
                    if "couter" in variant:
                        jc = [(j, c, s) for c in range(nc_eff)
                              for j in range(kt16) for s in range(nsplit)]
                    else:
                        jc = [(j, c, s) for j in range(kt16)
                              for c in range(nc_eff) for s in range(nsplit)]
                    for j, c, s in jc:
                        lhs = xt_sb[:, j, :]
                        rhs_slab = w8s[j // kth_full]
                        jj = (j % kth_full) % kth
                        st = (j == 0) or "allstart" in variant
                        sp = ((j == kt_eff - 1 and kt16 == kt_eff)
                              or "allstart" in variant)
                        off = c * O_CHUNK + s * oc
                        nc.tensor.matmul(
                            ps[c][:, s * oc:(s + 1) * oc], lhs,
                            rhs_slab[:, jj, off:off + oc],
                            start=st, stop=sp)
                    # fp8 DoubleRow tail: 2 k-tiles per matmul
                    if kt16 < kt_eff:
                        for jp in range(nk8 // 2):
                            j0 = kt16 + 2 * jp
                            rhs_slab = w8s[j0 // kth_full]
                            jj = j0 % kth_full
                            for c in range(N_OSL):
                                nc.tensor.matmul(
                                    ps[c][:], xt8f[:, jp, :, :],
                                    rhs_slab[:, jj:jj + 2,
                                             c * O_CHUNK:(c + 1) * O_CHUNK],
                                    start=False, stop=(jp == nk8 // 2 - 1),
                                    perf_mode=mybir.MatmulPerfMode.DoubleRow)
                else:
                    for c in range(N_OSL):
                        nc.tensor.matmul(
                            ps[c][:], xt_sb[:, 0, :],
                            w8s[0][:, 0, c * O_CHUNK:(c + 1) * O_CHUNK],
                            start=True, stop=True)
                for c in range(N_OSL):
                    sl = slice(c * O_CHUNK, (c + 1) * O_CHUNK)
                    nc.vector.tensor_add(out_sb[:, sl], ps[c][:],
                                         bias_bc[:, sl])
                # store on the ACT HWDGE ring (weight loads own the SP ring)
                store_eng = nc.sync if "storesp" in variant else nc.scalar
                store_eng.dma_start(o_d.ap(), out_sb[:])

            if repeats == 1:
                body(0)
            else:
                assert repeats % unroll == 0
                kw = {}
                if hint:
                    kw["hint_engines"] = (mybir.EngineType.PE,)
                if stag:
                    kw["staggered_reset"] = True
                with tc.For_i(0, repeats // unroll, 1, **kw):
                    for p in range(unroll):
                        body(p)

    nc.compile()
    if "nolwdedup" not in variant:
        _dedupe_ldweights(nc.m)
    return nc


def _dedupe_ldweights(m):
    """Drop InstLdweights that reload the exact weights already resident
    in the PE array. tile_legalize splits every self-loading InstMatmult
    into an Ldweights+Matmult pair, so 4 consecutive matmuls sharing one
    stationary x^T tile emit 4 identical loads; the extra 3 stall the PE
    ~88 cycles each (HW-measured; the load does not overlap the adjacent
    matmul). Keeps any load that carries semaphore waits/updates."""
    for fn in m.functions:
        for blk in fn.blocks:
            il = blk.instructions  # live list (writes through to rust)
            last_sig = None
            drop = []
            for idx, inst in enumerate(il):
                if str(inst.engine) != "EngineType.PE":
                    continue
                tn = type(inst).__name__
                if tn == "InstLdweights":
                    si = inst.sync_info
                    has_sync = bool(si and (getattr(si, "on_wait", None)
                                            or getattr(si, "on_update", None)))
                    sig = (str(inst.ins[0]), str(inst.perf_mode),
                           str(inst.is_transpose))
                    if sig == last_sig and not has_sync:
                        drop.append(idx)
                    else:
                        last_sig = sig
                elif tn == "InstMatmult":
                    pass  # streaming does not disturb the stationary array
                else:
                    last_sig = None  # conservative: anything else resets
            for idx in reversed(drop):
                il.pop(idx)


BEST_CONFIG = dict(wbufs=4, psum_bufs=8, unroll=12)


def _get_nc():
    global _NC
    if _NC is None:
        _NC = _build_nc(**BEST_CONFIG)
    return _NC


def make_per_core_inputs(x, weight_fp8, bias, nk8=NK8):
    """Host-side shard/layout prep shared by kernel() and the timing
    harness. Returns {name: array} with the per-core concatenated layout
    expected by the SPMD callable (axis 0 sharded over cores)."""
    x = np.ascontiguousarray(np.asarray(x), dtype=np.float32)
    w = np.ascontiguousarray(np.asarray(weight_fp8))
    b = np.ascontiguousarray(np.asarray(bias), dtype=np.float32)
    assert x.shape == (T, K) and w.shape == (O_FULL, K)

    # weight: per core c, wt[p, j*O + o] = w[c*O + o, j*128 + p]
    w8 = w.view(np.uint8)                       # [16384, 4096]
    wt = np.ascontiguousarray(
        w8.reshape(N_CORES, O, KT, 128).transpose(0, 3, 2, 1)
    ).reshape(N_CORES * 128, KT * O)

    # x^T: xt[p, j*T + t] = fp16(x[t, j*128 + p]); replicated per core
    x16 = x.astype(np.float16)                  # [128, 4096] (t, i)
    xt1 = np.ascontiguousarray(
        x16.reshape(T, KT, 128).transpose(2, 1, 0)).reshape(128, KT * T)
    xt = np.tile(xt1, (N_CORES, 1))             # [8*128, 4096]

    # fp8 x^T for the trailing NK8 k-tiles, DoubleRow plane-pair layout:
    # xt8[p, jp, r, t] = fp8(x[t, (KT-NK8 + 2*jp + r)*128 + p])
    import ml_dtypes
    nk = max(nk8, 2)
    x8 = x[:, (KT - nk) * 128:].astype(ml_dtypes.float8_e4m3fn)
    xt81 = np.ascontiguousarray(
        x8.reshape(T, nk, 128).transpose(2, 1, 0)      # [p, jtile, t]
    ).view(np.uint8).reshape(128, nk * T)
    xt8 = np.tile(xt81, (N_CORES, 1))

    return {
        "wt": wt,
        "xt": xt,
        "xt8": xt8,
        "bias": b.astype(np.float16).reshape(N_CORES, O),
    }


_FN = None


def _get_fn():
    """Cache the jitted SPMD callable so repeat kernel() calls skip the
    ~1.3s of re-tracing that run_bass_kernel_spmd pays per invocation."""
    global _FN
    if _FN is not None:
        return _FN
    import jax
    from jax.sharding import Mesh, PartitionSpec, NamedSharding
    from jax.experimental.shard_map import shard_map
    from concourse import bass2jax as b2j

    nc = _get_nc()
    b2j.install_neuronx_cc_hook()
    pname = nc.partition_id_tensor.name if nc.partition_id_tensor else None
    in_names, out_names, out_avals = [], [], []
    for alloc in nc.m.functions[0].allocations:
        if not isinstance(alloc, mybir.MemoryLocationSet):
            continue
        name = alloc.memorylocations[0].name
        if alloc.kind == "ExternalInput":
            if name != pname:
                in_names.append(name)
        elif alloc.kind == "ExternalOutput":
            out_names.append(name)
            out_avals.append(jax.core.ShapedArray(
                tuple(alloc.tensor_shape), mybir.dt.np(alloc.dtype)))
    n_params, n_outs = len(in_names), len(out_avals)
    all_in = in_names + out_names + ([pname] if pname else [])

    def _body(*args):
        operands = list(args)
        if pname:
            operands.append(b2j.partition_id_tensor())
        outs = b2j._bass_exec_p.bind(
            *operands, out_avals=tuple(out_avals), in_names=tuple(all_in),
            out_names=tuple(out_names), lowering_input_output_aliases=(),
            sim_require_finite=True, sim_require_nnan=True, nc=nc)
        return tuple(outs)

    mesh = Mesh(np.asarray(jax.devices()[:N_CORES]), ("core",))
    fn = jax.jit(shard_map(_body, mesh=mesh,
                           in_specs=(PartitionSpec("core"),) * (n_params + n_outs),
                           out_specs=(PartitionSpec("core"),) * n_outs,
                           check_rep=False), keep_unused=True)
    sharding = NamedSharding(mesh, PartitionSpec("core"))
    _FN = (fn, in_names, out_avals, sharding)
    return _FN


def kernel(x, weight_fp8, bias):
    import jax

    fn, in_names, out_avals, sharding = _get_fn()
    per_core = make_per_core_inputs(x, weight_fp8, bias)
    dev_in = [jax.device_put(per_core[n], sharding) for n in in_names]
    dev_zero = [jax.device_put(
        np.zeros((N_CORES * a.shape[0], *a.shape[1:]), a.dtype), sharding)
        for a in out_avals]
    outs = fn(*dev_in, *dev_zero)
    res = np.asarray(jax.device_get(outs[0])).reshape(N_CORES, T, O)
    return np.concatenate(
        [res[c] for c in range(N_CORES)], axis=1).astype(np.float32)
